# revision 4
# baseline (speedup 1.0000x reference)
"""Trainium2 Bass kernel for: out = (x @ wsums.sum(0)) * (1.5 * 0.5).

x: [1024, 8192] f32, wsums: [32, 8192] f32 -> out: [1024, 1] f32.

Sharding across 8 NeuronCores (2D grid): KSHARD-way along the contraction
dim k x BSHARD-way along the batch dim.  Each core reads a 4MB x shard +
a small wsums k-slice, computes partial row-dots for its
(batch-block, k-slice), and the host sums the KSHARD k-partials per
batch-block and concatenates the batch-blocks (the unshard step for a
contraction-sharded dim).

Per-core device program:
  1. DMA wsums slice [32, KB] -> SBUF.
  2. PE matmul with an all-ones [32, 128] stationary: reduces the 32 group
     rows AND broadcasts the result across all 128 partitions in one shot,
     directly into PSUM -> w_bcast [128, KB] (no PSUM->SBUF copy needed).
  3. For each row-block: DMA x rows [128, KB] -> SBUF; DVE tensor_tensor
     multiply y = x * w_bcast (w read straight from PSUM); ACT activation
     (Copy, scale=SCALE) with accum_out -> per-partition dot products.
  4. DMA the [128, NBLK] accumulator block to DRAM.

The walrus build in this container encodes at most ONE semaphore wait per
instruction ("Too many sync wait commands") and cannot encode bass_isa raw
ISA ops (tensor_tensor_reduce / partition_* -> "ISA wrong length"), so:
  - only classic mybir ops are used (TensorTensor / Activation / Matmult),
  - compile_bir_kernel is wrapped with a BIR post-pass that moves excess
    waits onto preceding same-engine NoOp instructions.
"""

import json
import os

import numpy as np

import concourse.bass as bass
import concourse.bass2jax as bass2jax
import concourse.bass_utils as bass_utils
import concourse.mybir as mybir
from concourse.tile import TileContext

SCALE = 1.5 * 0.5
B, K, G = 1024, 8192, 32
N_CORES = 8
KSHARD = int(os.environ.get("KERNEL_KSHARD", "4"))  # cores along k
BSHARD = N_CORES // KSHARD                          # cores along batch
KB = K // KSHARD                                    # per-core k width
BB = B // BSHARD                                    # per-core rows
P = 128
NBLK = BB // P                                      # row-blocks per core
F32 = mybir.dt.float32

# Set by test.py to profile; results stashed in LAST_RESULTS.
TRACE = False
TRACE_KWARGS = {}
LAST_RESULTS = None

_built = None

# ---------------------------------------------------------------------------
# Workaround: this container's walrus encodes at most 1 sync wait per
# instruction.  Split longer on_wait lists onto preceding same-engine NoOps.
MAX_WAITS = 1
_orig_compile_bir_kernel = bass_utils.compile_bir_kernel


def _split_waits_in_bir(bir: dict) -> int:
    counter = [0]

    def fix_blocks(blocks):
        for bb in blocks:
            out = []
            for ins in bb.get("instructions", []):
                si = ins.get("sync_info")
                ow = (si or {}).get("on_wait") or []
                if len(ow) > MAX_WAITS:
                    extra, keep = ow[:-MAX_WAITS], ow[-MAX_WAITS:]
                    for i in range(0, len(extra), MAX_WAITS):
                        counter[0] += 1
                        out.append({
                            "name": f"I-waitsplit-{counter[0]}",
                            "engine": ins["engine"],
                            "opcode": "NoOp",
                            "ins": [],
                            "outs": [],
                            "debug": ins.get("debug", 0),
                            "sync_info": {
                                "on_update": [],
                                "on_wait": extra[i : i + MAX_WAITS],
                            },
                        })
                    si["on_wait"] = keep
                out.append(ins)
            bb["instructions"] = out
            if bb.get("blocks"):
                fix_blocks(bb["blocks"])

    for fn in bir["functions"]:
        fix_blocks(fn["blocks"])
    return counter[0]


def _patched_compile_bir_kernel(bir_json, tmpdir, neff_name="file.neff"):
    if isinstance(bir_json, str):
        bir_json = bir_json.encode()
    bir = json.loads(bir_json)
    _split_waits_in_bir(bir)
    return _orig_compile_bir_kernel(json.dumps(bir).encode(), tmpdir, neff_name)


bass_utils.compile_bir_kernel = _patched_compile_bir_kernel
bass2jax.compile_bir_kernel = _patched_compile_bir_kernel
# ---------------------------------------------------------------------------


def _build():
    nc = bass.Bass("TRN2")
    x_sh = nc.dram_tensor("x_shard", (BB, KB), F32, kind="ExternalInput")
    w_sh = nc.dram_tensor("wsums_shard", (G, KB), F32, kind="ExternalInput")
    out = nc.dram_tensor("out_acc", (P, NBLK), F32, kind="ExternalOutput")

    with TileContext(nc) as tc:
        with (
            tc.tile_pool(name="const", bufs=1) as cpool,
            tc.tile_pool(name="xbuf", bufs=3) as xpool,
            tc.tile_pool(name="ybuf", bufs=2) as ypool,
            tc.tile_pool(name="psum", bufs=1, space="PSUM") as ppool,
        ):
            ws = cpool.tile([G, KB], F32)
            nc.sync.dma_start(out=ws, in_=w_sh.ap())

            ones = cpool.tile([G, P], F32)
            nc.vector.memset(ones, 1.0)

            # wp[m, n] = sum_g ones[g, m] * ws[g, n] = w_total[n] on every
            # partition m.  N<=512 per matmul (one PSUM bank each).
            wp = ppool.tile([P, KB], F32)
            for j in range(KB // 512):
                nc.tensor.matmul(
                    wp[:, j * 512 : (j + 1) * 512],
                    ones,
                    ws[:, j * 512 : (j + 1) * 512],
                    start=True,
                    stop=True,
                )

            acc = cpool.tile([P, NBLK], F32)
            xap = x_sh.ap()
            for r in range(NBLK):
                xt = xpool.tile([P, KB], F32, tag="xt")
                nc.sync.dma_start(out=xt, in_=xap[r * P : (r + 1) * P, :])
                yt = ypool.tile([P, KB], F32, tag="yt")
                nc.vector.tensor_tensor(yt, xt, wp, op=mybir.AluOpType.mult)
                nc.scalar.activation(
                    yt,
                    yt,
                    mybir.ActivationFunctionType.Copy,
                    scale=SCALE,
                    accum_out=acc[:, r : r + 1],
                )

            nc.sync.dma_start(out=out.ap(), in_=acc)
    return nc


def kernel(x: np.ndarray, wsums: np.ndarray) -> np.ndarray:
    global _built, LAST_RESULTS
    if _built is None:
        _built = _build()
    nc = _built

    in_maps = []
    for c in range(N_CORES):
        bb_i, kb_i = divmod(c, KSHARD)
        xs = np.ascontiguousarray(x[bb_i * BB : (bb_i + 1) * BB, kb_i * KB : (kb_i + 1) * KB])
        wsl = np.ascontiguousarray(wsums[:, kb_i * KB : (kb_i + 1) * KB])
        in_maps.append({"x_shard": xs, "wsums_shard": wsl})

    res = bass_utils.run_bass_kernel_spmd(
        nc,
        in_maps,
        core_ids=list(range(N_CORES)),
        trace=TRACE,
        **TRACE_KWARGS,
    )
    LAST_RESULTS = res

    parts = []
    for bb_i in range(BSHARD):
        tot = None
        for kb_i in range(KSHARD):
            acc = res.results[bb_i * KSHARD + kb_i]["out_acc"]  # [P, NBLK]
            vec = acc.T.reshape(BB)  # row 128*j + p  <-  acc[p, j]
            tot = vec if tot is None else tot + vec
        parts.append(tot)
    return np.concatenate(parts).astype(np.float32)[:, None]


# revision 8
# speedup vs baseline: 1.2013x; 1.2013x over previous
"""Trainium2 Bass kernel for: out = (x @ wsums.sum(0)) * (1.5 * 0.5).

x: [1024, 8192] f32, wsums: [32, 8192] f32 -> out: [1024, 1] f32.

Sharding across 8 NeuronCores (2D grid): KSHARD-way along the contraction
dim k x BSHARD-way along the batch dim.  Each core reads a 4MB x shard +
a small wsums k-slice, computes partial row-dots for its
(batch-block, k-slice), and the host sums the KSHARD k-partials per
batch-block and concatenates the batch-blocks (the unshard step for a
contraction-sharded dim).

Per-core device program:
  1. DMA wsums slice [32, KB] -> SBUF.
  2. PE matmul with an all-ones [32, 128] stationary: reduces the 32 group
     rows AND broadcasts the result across all 128 partitions in one shot,
     directly into PSUM -> w_bcast [128, KB] (no PSUM->SBUF copy needed).
  3. For each row-block: DMA x rows [128, KB] -> SBUF; DVE tensor_tensor
     multiply y = x * w_bcast (w read straight from PSUM); ACT activation
     (Copy, scale=SCALE) with accum_out -> per-partition dot products.
  4. DMA the [128, NBLK] accumulator block to DRAM.

The walrus build in this container encodes at most ONE semaphore wait per
instruction ("Too many sync wait commands") and cannot encode bass_isa raw
ISA ops (tensor_tensor_reduce / partition_* -> "ISA wrong length"), so:
  - only classic mybir ops are used (TensorTensor / Activation / Matmult),
  - compile_bir_kernel is wrapped with a BIR post-pass that moves excess
    waits onto preceding same-engine NoOp instructions.
"""

import json
import os

import numpy as np

import concourse.bass as bass
import concourse.bass2jax as bass2jax
import concourse.bass_utils as bass_utils
import concourse.mybir as mybir
from concourse.tile import TileContext

SCALE = 1.5 * 0.5
B, K, G = 1024, 8192, 32
N_CORES = 8
KSHARD = int(os.environ.get("KERNEL_KSHARD", "8"))  # cores along k
BSHARD = N_CORES // KSHARD                          # cores along batch
KB = K // KSHARD                                    # per-core k width
BB = B // BSHARD                                    # per-core rows
P = 128
NBLK = BB // P                                      # row-blocks per core
F32 = mybir.dt.float32

# Set by test.py to profile; results stashed in LAST_RESULTS.
TRACE = False
TRACE_KWARGS = {}
LAST_RESULTS = None

_built = None

# ---------------------------------------------------------------------------
# Workaround: this container's walrus encodes at most 1 sync wait per
# instruction.  Split longer on_wait lists onto preceding same-engine NoOps.
MAX_WAITS = 1
_orig_compile_bir_kernel = bass_utils.compile_bir_kernel


def _split_waits_in_bir(bir: dict) -> int:
    counter = [0]

    def fix_blocks(blocks):
        for bb in blocks:
            out = []
            for ins in bb.get("instructions", []):
                si = ins.get("sync_info")
                ow = (si or {}).get("on_wait") or []
                if len(ow) > MAX_WAITS:
                    extra, keep = ow[:-MAX_WAITS], ow[-MAX_WAITS:]
                    for i in range(0, len(extra), MAX_WAITS):
                        counter[0] += 1
                        out.append({
                            "name": f"I-waitsplit-{counter[0]}",
                            "engine": ins["engine"],
                            "opcode": "NoOp",
                            "ins": [],
                            "outs": [],
                            "debug": ins.get("debug", 0),
                            "sync_info": {
                                "on_update": [],
                                "on_wait": extra[i : i + MAX_WAITS],
                            },
                        })
                    si["on_wait"] = keep
                out.append(ins)
            bb["instructions"] = out
            if bb.get("blocks"):
                fix_blocks(bb["blocks"])

    for fn in bir["functions"]:
        fix_blocks(fn["blocks"])
    return counter[0]


def _patched_compile_bir_kernel(bir_json, tmpdir, neff_name="file.neff"):
    if isinstance(bir_json, str):
        bir_json = bir_json.encode()
    bir = json.loads(bir_json)
    _split_waits_in_bir(bir)
    return _orig_compile_bir_kernel(json.dumps(bir).encode(), tmpdir, neff_name)


bass_utils.compile_bir_kernel = _patched_compile_bir_kernel
bass2jax.compile_bir_kernel = _patched_compile_bir_kernel
# ---------------------------------------------------------------------------


def _build():
    nc = bass.Bass("TRN2")
    x_sh = nc.dram_tensor("x_shard", (BB, KB), F32, kind="ExternalInput")
    w_sh = nc.dram_tensor("wsums_shard", (G, KB), F32, kind="ExternalInput")
    out = nc.dram_tensor("out_acc", (P, NBLK), F32, kind="ExternalOutput")

    with TileContext(nc) as tc:
        with (
            tc.tile_pool(name="const", bufs=1) as cpool,
            tc.tile_pool(name="xbuf", bufs=max(1, NBLK // (2 if NBLK % 2 == 0 else 1))) as xpool,
            tc.tile_pool(name="ybuf", bufs=2) as ypool,
            tc.tile_pool(name="psum", bufs=1, space="PSUM") as ppool,
        ):
            ws = cpool.tile([G, KB], F32)
            nc.sync.dma_start(out=ws, in_=w_sh.ap())

            ones = cpool.tile([G, P], F32)
            nc.vector.memset(ones, 1.0)

            # wp[m, n] = sum_g ones[g, m] * ws[g, n] = w_total[n] on every
            # partition m.  N<=512 per matmul (one PSUM bank each).
            wp = ppool.tile([P, KB], F32)
            for j in range(KB // 512):
                nc.tensor.matmul(
                    wp[:, j * 512 : (j + 1) * 512],
                    ones,
                    ws[:, j * 512 : (j + 1) * 512],
                    start=True,
                    stop=True,
                )

            acc = cpool.tile([P, NBLK], F32)
            xap = x_sh.ap()
            # 2 row-blocks per DMA (1MB each); all chunk tiles resident so
            # every DMA is triggered up-front and streams at full rate.
            RB_PER_CHUNK = 2 if NBLK % 2 == 0 else 1
            NCHUNK = NBLK // RB_PER_CHUNK
            xtiles = []
            for j in range(NCHUNK):
                xt = xpool.tile([P, RB_PER_CHUNK * KB], F32, tag="xt")
                # src[p, a, k] = x_shard[j*RB*P + a*P + p, k]
                src = bass.AP(
                    x_sh,
                    j * RB_PER_CHUNK * P * KB,
                    [[KB, P], [P * KB, RB_PER_CHUNK], [1, KB]],
                )
                nc.sync.dma_start(out=xt, in_=src)
                xtiles.append(xt)
            for r in range(NBLK):
                j, a = divmod(r, RB_PER_CHUNK)
                xslice = xtiles[j][:, a * KB : (a + 1) * KB]
                yt = ypool.tile([P, KB], F32, tag="yt")
                nc.vector.tensor_tensor(yt, xslice, wp, op=mybir.AluOpType.mult)
                nc.scalar.activation(
                    yt,
                    yt,
                    mybir.ActivationFunctionType.Copy,
                    scale=SCALE,
                    accum_out=acc[:, r : r + 1],
                )

            nc.sync.dma_start(out=out.ap(), in_=acc)
    return nc


def kernel(x: np.ndarray, wsums: np.ndarray) -> np.ndarray:
    global _built, LAST_RESULTS
    if _built is None:
        _built = _build()
    nc = _built

    in_maps = []
    for c in range(N_CORES):
        bb_i, kb_i = divmod(c, KSHARD)
        xs = np.ascontiguousarray(x[bb_i * BB : (bb_i + 1) * BB, kb_i * KB : (kb_i + 1) * KB])
        wsl = np.ascontiguousarray(wsums[:, kb_i * KB : (kb_i + 1) * KB])
        in_maps.append({"x_shard": xs, "wsums_shard": wsl})

    res = bass_utils.run_bass_kernel_spmd(
        nc,
        in_maps,
        core_ids=list(range(N_CORES)),
        trace=TRACE,
        **TRACE_KWARGS,
    )
    LAST_RESULTS = res

    parts = []
    for bb_i in range(BSHARD):
        tot = None
        for kb_i in range(KSHARD):
            acc = res.results[bb_i * KSHARD + kb_i]["out_acc"]  # [P, NBLK]
            vec = acc.T.reshape(BB)  # row 128*j + p  <-  acc[p, j]
            tot = vec if tot is None else tot + vec
        parts.append(tot)
    return np.concatenate(parts).astype(np.float32)[:, None]
